# revision 1
# baseline (speedup 1.0000x reference)
"""Trainium2 Bass kernel for GeneralNonLinearReadoutBlock (gated equivariant MLP).

Reference computation (per node, fp32):
    x0 = x[:, :128]                 # scalars
    x1 = x[:, 128:].reshape(128,3)  # vectors, channel-major interleave (u,i)
    s  = x0 @ w1_s * c              # [256] -> (scalars | gates)
    v  = x1^T_i @ w1_v * c          # per component i
    h0 = silu(s[:128]); g = silu(s[128:])
    h1_i = v_i * g
    y0 = h0 @ w2_s * c ; y1_i = h1_i @ w2_v * c
    y  = concat(y0, interleave(y1))           c = 1/sqrt(128)

Strategy: data-parallel over nodes on 8 cores; the 1/sqrt(128) factors are
folded into the weights host-side.  Per 512-node macrotile, transpose the
activations once on the PE (feature-major), run linear-1 with the weights
stationary (moving dim 512), gate on ACT/DVE, then run linear-2 with the
*activations* stationary so the output comes back in natural [node, feat]
orientation with no second transpose.
"""

import sys

sys.path.insert(0, "/opt/trn_rl_repo")

import numpy as np

import concourse.bass as bass
import concourse.tile as tile
from concourse import masks, mybir
from concourse._compat import not_none as nn
from concourse.vector_clock import ScopedClock

MUL = 128
N_FULL = 100000
N_CORES = 8
ROWS_PER_CORE = 12544  # 98 tiles of 128; 8*12544 = 100352 (pad 352 rows)
F = 4 * MUL  # 512 features
INV = np.float32(1.0 / np.sqrt(np.float32(MUL)))

FP32 = mybir.dt.float32
FP32R = mybir.dt.float32r
BF16 = mybir.dt.bfloat16

# --- tunables (env-overridable for A/B experiments) -----------------------
import os as _os

MACRO = int(_os.environ.get("KOPT_MACRO", "512"))
MM1_FP32R = _os.environ.get("KOPT_MM1", "fp32r") == "fp32r"
MM2_DTYPE = _os.environ.get("KOPT_MM2", "fp32")   # "fp32" | "bf16"
XIN_BUFS = int(_os.environ.get("KOPT_XIN_BUFS", "4"))
YOUT_BUFS = int(_os.environ.get("KOPT_YOUT_BUFS", "4"))
XT_BUFS = int(_os.environ.get("KOPT_XT_BUFS", "10"))
H_BUFS = int(_os.environ.get("KOPT_H_BUFS", "12"))


class SplitDrainTileContext(tile.TileContext):
    """TileContext whose final drain splits sem waits across SP nops.

    The pinned walrus rejects >1 sync-wait on a TPB_CTRL drain; stock
    TileContext puts every outstanding proc's wait on the one tail drain.
    """

    MAXW = 1

    def _split_waits_everywhere(self):
        """Ensure no instruction carries more than MAXW sem waits by moving
        excess waits onto same-engine nops inserted just before it."""
        nc = self.nc
        cur = nn(nc.cur_bb).bb
        eng_map = {
            mybir.EngineType.PE: nc.tensor,
            mybir.EngineType.DVE: nc.vector,
            mybir.EngineType.Activation: nc.scalar,
            mybir.EngineType.Pool: nc.gpsimd,
            mybir.EngineType.SP: nc.sync,
        }
        for f in nc.m.functions:
            for bb in f.blocks:
                new_insts = []
                changed = False
                for inst in bb.instructions:
                    si = inst.sync_info
                    waits = list(si.on_wait) if si is not None else []
                    if len(waits) > self.MAXW:
                        changed = True
                        chunks = [
                            waits[i : i + self.MAXW]
                            for i in range(0, len(waits), self.MAXW)
                        ]
                        for chunk in chunks[:-1]:
                            nop = eng_map[inst.engine].nop(
                                nofuse=True, hint="wait_split"
                            )
                            assert cur.instructions[-1] is nop.ins
                            cur.instructions.pop()
                            nop.ins.sync_info = mybir.SyncInfo(
                                on_wait=chunk, on_update=[]
                            )
                            new_insts.append(nop.ins)
                        si.on_wait = chunks[-1]
                        inst.sync_info = si
                    new_insts.append(inst)
                if changed:
                    bb.instructions[:] = new_insts

    def _drain_and_barrier(self, tick_clock, wait_clock):
        self._split_waits_everywhere()
        drain_inst = self.nc.sync.drain()
        wait_clock.add_sem_waits(
            drain_inst.ins, ScopedClock({None: tick_clock.global_clock})
        )
        waits = list(drain_inst.ins.sync_info.on_wait)
        if len(waits) > self.MAXW:
            chunks = [waits[i : i + self.MAXW] for i in range(0, len(waits), self.MAXW)]
            si = drain_inst.ins.sync_info
            si.on_wait = chunks[-1]
            drain_inst.ins.sync_info = si
            bb = nn(self.nc.cur_bb).bb
            assert bb.instructions[-1] is drain_inst.ins
            bb.instructions.pop()
            for chunk in chunks[:-1]:
                nop = self.nc.sync.nop(nofuse=True, hint="drain_wait_split")
                nop.ins.sync_info = mybir.SyncInfo(on_wait=chunk, on_update=[])
            bb.instructions.append(drain_inst.ins)
        self.nc.all_engine_barrier()
        assert self.sems is not None
        popped = self.nc._tile_sem_poison_stack.pop()
        assert popped is self._sem_poison
        self.nc.clear_and_free_semaphores(list(self.sems.allocated().values()))
        self.nc.all_engine_barrier()


def build_ir(tc, y_d, x_d, w1s_d, w1v_d, w2s_d, w2v_d, n_rows, repeats=1):
    """Emit the per-core kernel IR. n_rows must be a multiple of 128."""
    nc = tc.nc
    assert n_rows % 128 == 0
    n_tiles = n_rows // 128
    # macrotile sizes (in 128-row subtiles)
    SM = MACRO // 128
    macros = [SM] * (n_tiles // SM)
    if n_tiles % SM:
        macros.append(n_tiles % SM)

    mm2_dt = {"fp32": FP32, "bf16": BF16}[MM2_DTYPE]
    ident_dt = {"bf16": BF16, "fp32": FP32}[_os.environ.get("KOPT_IDENT", "fp32")]

    with (
        tc.tile_pool(name="consts", bufs=1) as consts,
        tc.tile_pool(name="xin", bufs=XIN_BUFS) as xin_pool,
        tc.tile_pool(name="xt", bufs=XT_BUFS) as xt_pool,
        tc.tile_pool(name="h", bufs=H_BUFS) as h_pool,
        tc.tile_pool(name="yout", bufs=YOUT_BUFS) as yout_pool,
        tc.tile_pool(name="tpp", bufs=2, space="PSUM") as tp_psum,
        tc.tile_pool(name="ps_s", bufs=2, space="PSUM") as s_psum,
        tc.tile_pool(name="ps_v", bufs=2, space="PSUM") as v_psum,
        tc.tile_pool(name="ps_y", bufs=2, space="PSUM") as y_psum,
    ):
        # ---- constants: identity + weights (pre-scaled host-side) --------
        mm1_dt = FP32R if MM1_FP32R else FP32
        ident = consts.tile([128, 128], ident_dt)
        masks.make_identity(nc, ident[:])
        w1s = consts.tile([128, 2 * MUL], mm1_dt)
        w1v = consts.tile([128, MUL], mm1_dt)
        if mm1_dt == FP32:
            nc.sync.dma_start(w1s[:], w1s_d[:, :])
            nc.sync.dma_start(w1v[:], w1v_d[:, :])
        else:
            w1s_f32 = consts.tile([128, 2 * MUL], FP32)
            w1v_f32 = consts.tile([128, MUL], FP32)
            nc.sync.dma_start(w1s_f32[:], w1s_d[:, :])
            nc.sync.dma_start(w1v_f32[:], w1v_d[:, :])
            nc.vector.tensor_copy(w1s[:], w1s_f32[:])
            nc.vector.tensor_copy(w1v[:], w1v_f32[:])
        w2s = consts.tile([128, MUL], mm2_dt)
        w2v = consts.tile([128, MUL], mm2_dt)
        if mm2_dt == FP32:
            nc.sync.dma_start(w2s[:], w2s_d[:, :])
            nc.sync.dma_start(w2v[:], w2v_d[:, :])
        else:
            w2s_f32 = consts.tile([128, MUL], FP32)
            w2v_f32 = consts.tile([128, MUL], FP32)
            nc.sync.dma_start(w2s_f32[:], w2s_d[:, :])
            nc.sync.dma_start(w2v_f32[:], w2v_d[:, :])
            nc.vector.tensor_copy(w2s[:], w2s_f32[:])
            nc.vector.tensor_copy(w2v[:], w2v_f32[:])


        for _rep in range(repeats):
            _run_macro_loop(
                nc, tc, macros, y_d, x_d, xin_pool, xt_pool, h_pool, yout_pool,
                tp_psum, s_psum, v_psum, y_psum, ident, w1s, w1v, w2s, w2v,
                mm1_dt, mm2_dt,
            )


def _run_macro_loop(nc, tc, macros, y_d, x_d, xin_pool, xt_pool, h_pool,
                    yout_pool, tp_psum, s_psum, v_psum, y_psum, ident,
                    w1s, w1v, w2s, w2v, mm1_dt, mm2_dt):
    if True:
        r0 = 0
        for S in macros:
            nf = S * 128  # moving/free dim for this macrotile
            rows = S * 128

            # ---- load [rows, 512] as one contiguous DMA ------------------
            xin = xin_pool.tile([128, S, F], FP32, tag="xin")
            src = x_d[r0 : r0 + rows, :].rearrange("(s p) f -> p s f", p=128)
            nc.sync.dma_start(xin[:], src)

            # ---- transposes: [n,f]-major -> [f,n]-major ------------------
            # group 0: scalars x0; groups 1..3: vector component i
            xt = []
            for gidx in range(4):
                pt = tp_psum.tile([128, nf], FP32, tag="tpp")
                for s in range(S):
                    if gidx == 0:
                        src_ap = xin[:, s, 0:MUL]
                    else:
                        src_ap = xin[:, s, MUL:].rearrange(
                            "p (u three) -> p u three", three=3
                        )[:, :, gidx - 1]
                    nc.tensor.transpose(
                        pt[:, s * 128 : (s + 1) * 128], src_ap, ident[:]
                    )
                st = xt_pool.tile([128, nf], mm1_dt, tag="xt")
                # alternate ACT/DVE for the psum->sbuf copies
                if gidx % 2 == 0:
                    nc.scalar.copy(st[:], pt[:])
                else:
                    nc.vector.tensor_copy(st[:], pt[:])
                xt.append(st)

            # ---- linear 1 (weights stationary, activations moving) -------
            ps_a = s_psum.tile([128, nf], FP32, tag="ps_s")
            nc.tensor.matmul(
                ps_a[:], w1s[:, 0:MUL], xt[0][:], start=True, stop=True
            )
            ps_b = s_psum.tile([128, nf], FP32, tag="ps_s")
            nc.tensor.matmul(
                ps_b[:], w1s[:, MUL:], xt[0][:], start=True, stop=True
            )
            ps_v = []
            for i in range(3):
                pv = v_psum.tile([128, nf], FP32, tag="ps_v")
                nc.tensor.matmul(
                    pv[:], w1v[:], xt[1 + i][:], start=True, stop=True
                )
                ps_v.append(pv)

            # ---- gate ----------------------------------------------------
            h0 = h_pool.tile([128, nf], mm2_dt, tag="h")
            nc.scalar.activation(h0[:], ps_a[:], mybir.ActivationFunctionType.Silu)
            g = h_pool.tile([128, nf], FP32, tag="h")
            nc.scalar.activation(g[:], ps_b[:], mybir.ActivationFunctionType.Silu)
            h1 = []
            for i in range(3):
                hi = h_pool.tile([128, nf], mm2_dt, tag="h")
                nc.vector.tensor_mul(hi[:], ps_v[i][:], g[:])
                h1.append(hi)

            # ---- linear 2 (activations stationary -> natural layout) -----
            yout = yout_pool.tile([128, S, F], FP32, tag="yout")
            for pidx, (act, w2) in enumerate(
                [(h0, w2s), (h1[0], w2v), (h1[1], w2v), (h1[2], w2v)]
            ):
                py = y_psum.tile([128, nf], FP32, tag="ps_y")
                for j in range(S):
                    nc.tensor.matmul(
                        py[:, j * 128 : (j + 1) * 128],
                        act[:, j * 128 : (j + 1) * 128],
                        w2[:],
                        start=True,
                        stop=True,
                    )
                if pidx == 0:
                    dst = yout[:, :, 0:MUL]
                else:
                    dst = yout[:, :, MUL:].rearrange(
                        "p s (u three) -> p s u three", three=3
                    )[:, :, :, pidx - 1]
                src_ap = py[:].rearrange("p (s n) -> p s n", s=S)
                # alternate ACT/DVE on output copies (ACT is lighter loaded)
                if pidx in (0, 1):
                    nc.scalar.copy(dst, src_ap)
                else:
                    nc.vector.tensor_copy(dst, src_ap)

            # ---- store ---------------------------------------------------
            dst = y_d[r0 : r0 + rows, :].rearrange("(s p) f -> p s f", p=128)
            nc.scalar.dma_start(dst, yout[:])

            r0 += rows


def build_bass(n_rows=ROWS_PER_CORE, repeats=1):
    nc = bass.Bass(trn_type="TRN2", target_bir_lowering=False, debug=False)
    x_d = nc.dram_tensor("x", [n_rows, F], FP32, kind="ExternalInput").ap()
    w1s_d = nc.dram_tensor("w1_s", [MUL, 2 * MUL], FP32, kind="ExternalInput").ap()
    w1v_d = nc.dram_tensor("w1_v", [MUL, MUL], FP32, kind="ExternalInput").ap()
    w2s_d = nc.dram_tensor("w2_s", [MUL, MUL], FP32, kind="ExternalInput").ap()
    w2v_d = nc.dram_tensor("w2_v", [MUL, MUL], FP32, kind="ExternalInput").ap()
    y_d = nc.dram_tensor("y", [n_rows, F], FP32, kind="ExternalOutput").ap()
    with SplitDrainTileContext(nc) as tc:
        build_ir(tc, y_d, x_d, w1s_d, w1v_d, w2s_d, w2v_d, n_rows, repeats=repeats)
    return nc


def shard_inputs(x, w1_s, w1_v, w2_s, w2_v):
    """Pad + shard x row-wise; pre-scale weights by 1/sqrt(128)."""
    x = np.ascontiguousarray(np.asarray(x, dtype=np.float32))
    pad = N_CORES * ROWS_PER_CORE - x.shape[0]
    if pad:
        x = np.concatenate([x, np.zeros((pad, x.shape[1]), np.float32)], axis=0)
    shards = x.reshape(N_CORES, ROWS_PER_CORE, F)
    w = {
        "w1_s": np.asarray(w1_s, np.float32) * INV,
        "w1_v": np.asarray(w1_v, np.float32) * INV,
        "w2_s": np.asarray(w2_s, np.float32) * INV,
        "w2_v": np.asarray(w2_v, np.float32) * INV,
    }
    return [dict(w, x=np.ascontiguousarray(shards[c])) for c in range(N_CORES)]


_NC_CACHE = {}


def kernel(x, w1_s, w1_v, w2_s, w2_v):
    from concourse.bass_utils import run_bass_kernel_spmd

    # building + Tile-scheduling the module costs ~10s of host CPU; reuse it
    # (the module is read-only after construction) across repeated calls.
    if "nc" not in _NC_CACHE:
        _NC_CACHE["nc"] = build_bass()
    nc = _NC_CACHE["nc"]
    in_maps = shard_inputs(x, w1_s, w1_v, w2_s, w2_v)
    res = run_bass_kernel_spmd(nc, in_maps, core_ids=list(range(N_CORES)))
    y = np.concatenate([res.results[c]["y"] for c in range(N_CORES)], axis=0)
    return y[:N_FULL]

